# revision 20
# baseline (speedup 1.0000x reference)
"""KoLeo loss kernel for Trainium2 (8 NeuronCores, data-parallel rows).

reference semantics:
    x = l2_normalize(student_output)            # [B, D]
    dots = x @ x.T ; dots[i, i] = -1
    I = argmax(dots, 1)
    loss = -mean(log(||x - x[I] + eps|| + eps))

Since rows are unit-norm, ||x_i - x_j|| = sqrt(2 - 2 * dot(x_i, x_j)), so
    loss = -0.5 * mean(ln(2 - 2 * max_j!=i dots[i, j]))
(the eps terms contribute ~1e-8 relative and are dropped).

Sharding: each core gets the full x^T (bf16, host-cast), column-rotated so its
own 1024 rows come first, computes its [1024, 8192] slice of the gram matrix
in fp8-e4m3 with DoubleRow matmuls (2 K-planes per instruction), and reduces
to a scalar partial sum of ln(2 - 2*maxdot).  Host sums the 8 partials.

Engine budget per core (all three within ~2us of each other, ~81us busy):
  PE : 256 DR gram matmuls + 32 presummed norm matmuls + diag-kill + warmup
  DVE: xsq + presum adds + bf16*bf16->fp8 normalize muls (fp8-out is ~2x
       slower than bf16-out; fp8 *reads* are also ~2x, so the input stays
       bf16) + 14 exact reduce_max drains
  ACT: 52 fused Exp smooth-max drains (accum_out row sums) + ln/exp norms

Hard-won scheduling rules baked in below:
  - GpSimd is NEVER used for tensor work: any big GP op degrades concurrent
    DVE throughput ~2.5x (measured 1136 -> 2830 ns per [128,1024] mul).
  - xsq pairs are presummed on DVE so the column-norm ones-matmul contracts
    2 planes instead of 4 (PE 17us -> 4us).
  - Direct (exact reduce_max, DVE) drains are tiles mi<2 of groups 3-7
    plus mi>=4 of the last group: spread evenly across group windows, early
    in each group so they never queue behind the next group's normalize
    muls in the DVE FIFO, and the closing drain wave lands on idle DVE.
  - 24+24 identity warmup matmuls bracket norm_stage(0): PE pstate drops to
    half clock after ~3us idle, and the first gram otherwise pays 2x.
  - group-0 input DMA is split into [32,1024] slices across all 16 queues
    (descriptor-rate-bound), the rest ride as 2048-wide 2KB-granule slices.

Other structure per 1024-wide column group:
  1. column norms via bf16 ones-matmul (PE broadcasts sums across
     partitions); inv = exp(-0.5*ln(norm2)) (ACT; the act-table universe is
     pinned to natural_log_exp_and_others => exactly one ACT_TABLE_LOAD)
  2. normalize: xq = x * inv, bf16*bf16 -> fp8 DoubleRow planes (DVE)
  3. gram: per row-tile one [128,1024] PSUM tile, 4 DoubleRow matmuls
     (K=256 each); diagonal killed by one extra bf16 I.T@(-64 shifted)
  4. drain: ACT smooth-max exp(beta*(dot-c)) with accum_out row sums
     (beta=256, c=0.22: error ~1e-4 vs exact; exp(256*(-64-c)) flushes the
     diagonal to zero for free), except the 12 direct-DVE tiles; epilogue
     folds smooth+direct maxes, ln(2-2*max) with a fused accum_out row-sum,
     then a partition-sum via f32 ones-matmul; the epilogue is split by
     row-tile halves so half A overlaps the last drain wave (host sums the
     [1,2] partials)
"""

import numpy as np
import ml_dtypes

import concourse.bacc as bacc
import concourse.hw_specs as hw_specs
import concourse.tile as tile
from concourse import mybir
from concourse.bass_utils import run_bass_kernel_spmd

B, D = 8192, 512
N_CORES = 8
ROWS = B // N_CORES          # 1024 rows per core
P = 128                      # SBUF partitions
KT = D // P                  # 4 contraction k-tiles
KB = KT // 2                 # 2 DoubleRow k-blocks (2 planes each)
M_TILES = ROWS // P          # 8 output row tiles
NT = 512                     # matmul moving free dim (psum bank)
GW = 1024                    # column-group width == gram PSUM tile (2 banks)
NG = B // GW                 # 8 column groups
DIAG_C = 64.0                # diagonal kill constant
N_WARM = 36                  # PE warm-up matmuls issued under the input DMA

BETA = 256.0                 # smooth-max sharpness
CSHIFT = 0.22                # smooth-max recentering constant
# Drain engine per column group: 3 of 8 groups go to exact DVE reduce_max,
# the rest to the ACT smooth-max.  Whole-group assignment keeps each engine
# FIFO free of cross-group head-of-line blocking (per-tile interleaving was
# measured slower: ready drains queue behind muls that wait on the next
# group's Exp).
DIRECT_SET = (2, 3, 4, 5, 6, 7)  # see _drain_direct
# Last group: drains alternate DVE/ACT per row-tile so the final drain wave
# finishes ~2x sooner (nothing queues behind them, so no FIFO hazard).
N_DIRECT = 6
N_EXP = NG


def _drain_direct(mi, ng):
    # 2 direct tiles per group, EARLY in the group (before the next group's
    # normalize muls enter the DVE queue), across groups 3-7: spreads the
    # DVE drain load evenly; the last group's final two tiles also go direct
    # so the closing drain wave lands on the by-then-idle DVE, not the
    # backlogged ACT
    return (ng >= 3 and mi < 2) or (ng == NG - 1 and mi >= 4)


def _drain_idx(mi, ng):
    """Index of this (mi, ng) tile within its row's direct or exp slots."""
    kind = _drain_direct(mi, ng)
    return sum(
        1 for n in range(ng) if _drain_direct(mi, n) == kind
    )

F32 = mybir.dt.float32
BF16 = mybir.dt.bfloat16
FP8 = mybir.dt.float8e4
AF = mybir.ActivationFunctionType
ALU = mybir.AluOpType
DR = mybir.MatmulPerfMode.DoubleRow

_CACHE: dict = {}


def _pin_act_tables():
    """Restrict the activation-table universe to natural_log_exp_and_others
    (contains ln, exp, copy, square, identity) so the table-load inserter
    emits exactly one ACT_TABLE_LOAD instead of thrashing ln<->exp sets.
    Set positions are preserved so the emitted act_func_set_id still indexes
    act_info.json correctly."""
    orig = hw_specs.get_activation_tables("gen3")
    pinned = {
        name: (fns if name == "natural_log_exp_and_others" else set())
        for name, fns in orig.items()
    }
    bacc.get_activation_tables = lambda arch: pinned


def _build():
    _pin_act_tables()
    nc = bacc.Bacc(
        "TRN2", target_bir_lowering=False, debug=False, num_devices=N_CORES
    )
    xt = nc.declare_dram_parameter("xt", [D, B], BF16, isOutput=False)
    ident = nc.declare_dram_parameter("ident", [P, P], BF16, isOutput=False)
    # ebig[p, 384 + p] = -DIAG_C, zero elsewhere; slicing [384-off : 896-off]
    # yields a [P, NT] tile with -DIAG_C at [p, off + p]
    ebig = nc.declare_dram_parameter("ebig", [P, NT + 3 * P], BF16, isOutput=False)
    partial = nc.declare_dram_parameter("partial", [1, 2], F32, isOutput=True)

    with tile.TileContext(nc) as tc:
        with (
            tc.tile_pool(name="big", bufs=1) as big,
            tc.tile_pool(name="work", bufs=2) as work,
            tc.tile_pool(name="sq", bufs=4) as sqp,
            tc.tile_pool(name="scr", bufs=5) as scrp,
            tc.tile_pool(name="small", bufs=2) as small,
        ):
            ident_sb = big.tile([P, P], BF16, name="ident_sb", tag="ident_sb")
            ebig_sb = big.tile([P, NT + 3 * P], BF16, name="ebig_sb", tag="ebig_sb")
            ones_sb = big.tile([P, P], BF16, name="ones_sb", tag="ones_sb")
            onesf_sb = big.tile([P, 1], F32, name="onesf_sb", tag="onesf_sb")
            nc.sync.dma_start(ident_sb[:], ident[:])
            nc.sync.dma_start(ebig_sb[:], ebig[:])
            nc.gpsimd.memset(ones_sb[:], 1.0)
            nc.gpsimd.memset(onesf_sb[:], 1.0)
            two_sb = small.tile([P, 1], F32, name="two_sb", tag="two_sb")
            nc.gpsimd.memset(two_sb[:], 2.0)
            # exp bias: exp(BETA*dot - BETA*CSHIFT)
            ebias_sb = small.tile([P, 1], F32, name="ebias_sb", tag="ebias_sb")
            nc.gpsimd.memset(ebias_sb[:], -BETA * CSHIFT)

            x8 = [
                big.tile([P, B], BF16, name=f"x8{k}", tag=f"x8{k}")
                for k in range(KT)
            ]
            xq = [
                big.tile([P, 2, B], FP8, name=f"xq{kb}", tag=f"xq{kb}")
                for kb in range(KB)
            ]
            invb = big.tile([P, B], BF16, name="invb", tag="invb")
            loglist = small.tile([P, M_TILES], F32, name="loglist", tag="loglist")
            # per (mi, direct slot) row-max candidates; 3D so the epilogue
            # can reduce all row-tiles in one instruction
            maxall = small.tile(
                [P, M_TILES, N_DIRECT], F32, name="maxall", tag="maxall"
            )
            # per (mi, exp slot) partial exp-sums
            expall = small.tile(
                [P, M_TILES, N_EXP], F32, name="expall", tag="expall"
            )
            # throwaway broadcast target for the fused drain's elementwise out
            dump = small.tile([P, 1], BF16, name="dump", tag="dump")
            # the last-group drain split leaves some slots unwritten
            nc.gpsimd.memset(maxall[:], -30000.0)
            nc.gpsimd.memset(expall[:], 0.0)

            with (
                tc.tile_pool(name="npsum", bufs=1, space="PSUM") as npsum,
                tc.tile_pool(name="gpsum", bufs=3, space="PSUM") as gpsum,
            ):
                # group 0 split across 16 queues (fp8 rows are 1KB granules,
                # descriptor-rate-bound; parallelism compensates), the rest as
                # 2KB-granule 2048-wide slices
                for k in range(KT):
                    for q in range(2):
                        r = slice(k * P + 64 * q, k * P + 64 * q + 64)
                        nc.sync.dma_start(x8[k][64 * q : 64 * q + 64, 0:GW], xt[r, 0:GW])
                for g0 in (1, 3, 5):
                    for k in range(KT):
                        nc.sync.dma_start(
                            x8[k][:, g0 * GW : (g0 + 2) * GW],
                            xt[k * P : (k + 1) * P, g0 * GW : (g0 + 2) * GW],
                        )
                for k in range(KT):
                    nc.sync.dma_start(
                        x8[k][:, 7 * GW : 8 * GW], xt[k * P : (k + 1) * P, 7 * GW : 8 * GW]
                    )

                # PE warm-up: keep the HAM activity window busy during the
                # initial DMA so gram matmuls run at 2.4 GHz from the start.
                warm = gpsum.tile([P, GW], F32, name="warm", tag="g")
                for _ in range(N_WARM):
                    nc.tensor.matmul(
                        warm[:, 0:P], ident_sb[:], ident_sb[:], start=True, stop=True
                    )

                def norm_stage(ng):
                    """xsq -> ones-matmul -> ln/exp -> normalize for group ng.
                    Emitted one group ahead, mid-way through the previous
                    group's gram loop, so the PE FIFO interleaves the norm
                    matmuls with gram matmuls instead of serializing at the
                    group boundary."""
                    ns = slice(ng * GW, (ng + 1) * GW)
                    # GpSimd is kept OFF this path entirely: its slow
                    # [128,1024] ops degrade concurrent DVE throughput ~2.5x
                    xsq = [
                        sqp.tile([P, GW], BF16, name=f"xsq_{ng}_{k}", tag=f"xsq{k}")
                        for k in range(KT)
                    ]
                    for k in range(KT):
                        nc.vector.tensor_mul(xsq[k][:], x8[k][:, ns], x8[k][:, ns])
                    # pre-sum pairs so the ones-matmul contracts 2 planes not 4
                    x01 = sqp.tile([P, GW], BF16, name=f"x01_{ng}", tag="x01")
                    x23 = sqp.tile([P, GW], BF16, name=f"x23_{ng}", tag="x23")
                    nc.vector.tensor_add(x01[:], xsq[0][:], xsq[1][:])
                    nc.vector.tensor_add(x23[:], xsq[2][:], xsq[3][:])
                    nps = npsum.tile([P, GW], F32, name="nps", tag="nps")
                    for c in range(GW // NT):
                        cs = slice(c * NT, (c + 1) * NT)
                        nc.tensor.matmul(
                            nps[:, cs], ones_sb[:], x01[:, cs], start=True, stop=False
                        )
                        nc.tensor.matmul(
                            nps[:, cs], ones_sb[:], x23[:, cs], start=False, stop=True
                        )
                    # inv = exp(-0.5*ln(norm2)); one pinned table set
                    lntmp = work.tile([P, GW], F32, name="lntmp", tag="lntmp")
                    nc.scalar.activation(lntmp[:], nps[:], AF.Ln)
                    nc.scalar.activation(
                        invb[:, ns], lntmp[:], AF.Exp, scale=-0.5
                    )
                    # normalize into fp8 DoubleRow planes: xq = x * inv
                    # (single op; a bf16-mul + fp8-copy split was measured
                    # slower despite the fp8-out 1x rate)
                    for k in range(KT):
                        nc.vector.tensor_mul(
                            xq[k // 2][:, k % 2, ns], x8[k][:, ns], invb[:, ns]
                        )

                norm_stage(0)
                for _ in range(24):
                    nc.tensor.matmul(
                        warm[:, 0:P], ident_sb[:], ident_sb[:], start=True, stop=True
                    )
                for ng in range(NG):
                    # gram slice rows x this column group, then row-max drain
                    for mi in range(M_TILES):
                        if mi == 3 and ng + 1 < NG:
                            norm_stage(ng + 1)
                        g = gpsum.tile([P, GW], F32, name="g", tag="g")
                        # diag block for row-tile mi sits at columns
                        # [mi*128, mi*128+128) -- always group 0
                        diag_here = ng == 0
                        diag_c = (mi * P) // NT
                        for kb in range(KB):
                            for c in range(GW // NT):
                                c0 = ng * GW + c * NT
                                nc.tensor.matmul(
                                    g[:, c * NT : (c + 1) * NT],
                                    xq[kb][:, :, mi * P : (mi + 1) * P],
                                    xq[kb][:, :, c0 : c0 + NT],
                                    start=(kb == 0),
                                    stop=(
                                        kb == KB - 1
                                        and not (diag_here and c == diag_c)
                                    ),
                                    perf_mode=DR,
                                )
                        if diag_here:
                            off = (mi * P) % NT
                            # adds -DIAG_C at diag position [p, off+p]
                            nc.tensor.matmul(
                                g[:, diag_c * NT : (diag_c + 1) * NT],
                                ident_sb[:],
                                ebig_sb[:, 3 * P - off : 3 * P - off + NT],
                                start=False,
                                stop=True,
                            )
                        si = _drain_idx(mi, ng)
                        if _drain_direct(mi, ng):
                            nc.vector.reduce_max(
                                maxall[:, mi, si : si + 1],
                                g[:],
                                axis=mybir.AxisListType.X,
                            )
                        else:
                            # fused smooth-max drain on ACT: accum_out sums
                            # exp(BETA*(dot - CSHIFT)) along the row
                            scr = scrp.tile([P, GW], BF16, name="scr", tag="scr")
                            nc.scalar.activation(
                                scr[:],
                                g[:],
                                AF.Exp,
                                bias=ebias_sb[:],
                                scale=BETA,
                                accum_out=expall[:, mi, si : si + 1],
                            )

# --- epilogue, split by row-tile halves: mi 0-3 finish their last
                # drain ~3us before mi 4-7, so half A's serial chain overlaps
                # the closing drain wave; host sums the two partials.
                # S >= exp(beta*(min-max-dot-c)) ~ 1e-10 here, no clamp needed
                tot = npsum.tile([P, GW], F32, name="tot", tag="nps")
                for hh in range(2):
                    sl = slice(4 * hh, 4 * hh + 4)
                    acc = small.tile([P, 4], F32, name=f"acc{hh}", tag=f"acc{hh}")
                    nc.vector.reduce_sum(
                        acc[:], expall[:, sl, :], axis=mybir.AxisListType.X
                    )
                    lnacc = small.tile([P, 4], F32, name=f"lnacc{hh}", tag=f"lnacc{hh}")
                    nc.scalar.activation(lnacc[:], acc[:], AF.Ln)
                    smooth = small.tile([P, 4], F32, name=f"smooth{hh}", tag=f"smooth{hh}")
                    nc.vector.tensor_scalar(
                        smooth[:],
                        lnacc[:],
                        1.0 / BETA,
                        CSHIFT,
                        op0=ALU.mult,
                        op1=ALU.add,
                    )
                    dmax = small.tile([P, 4], F32, name=f"dmax{hh}", tag=f"dmax{hh}")
                    nc.vector.reduce_max(
                        dmax[:], maxall[:, sl, :], axis=mybir.AxisListType.X
                    )
                    rowmax = small.tile([P, 4], F32, name=f"rowmax{hh}", tag=f"rowmax{hh}")
                    nc.vector.tensor_max(rowmax[:], dmax[:], smooth[:])
                    sumlog = small.tile([P, 1], F32, name=f"sumlog{hh}", tag=f"sumlog{hh}")
                    nc.scalar.activation(
                        loglist[:, sl], rowmax[:], AF.Ln, bias=two_sb[:],
                        scale=-2.0, accum_out=sumlog[:],
                    )
                    nc.tensor.matmul(
                        tot[0:1, hh : hh + 1], sumlog[:], onesf_sb[:],
                        start=True, stop=True,
                    )
                part_sb = small.tile([1, 2], F32, name="part_sb", tag="part_sb")
                nc.vector.tensor_copy(part_sb[:], tot[0:1, 0:2])
                nc.sync.dma_start(partial[:], part_sb[:])

    nc.finalize()
    return nc


def _get_nc():
    if "nc" not in _CACHE:
        _CACHE["nc"] = _build()
    return _CACHE["nc"]


def _in_maps(x: np.ndarray) -> list[dict]:
    ident = np.eye(P, dtype=np.float32).astype(ml_dtypes.bfloat16)
    ebig = np.zeros((P, NT + 3 * P), dtype=np.float32)
    ebig[np.arange(P), 3 * P + np.arange(P)] = -DIAG_C
    ebig = ebig.astype(ml_dtypes.bfloat16)
    x8 = x.astype(ml_dtypes.bfloat16)
    maps = []
    for m in range(N_CORES):
        xrot = np.concatenate([x8[m * ROWS :], x8[: m * ROWS]], axis=0)
        maps.append(
            {
                "xt": np.ascontiguousarray(xrot.T),
                "ident": ident,
                "ebig": ebig,
            }
        )
    return maps


def run_kernel(x: np.ndarray, **spmd_kwargs):
    """Returns (loss_scalar_f32, BassKernelResults)."""
    res = run_bass_kernel_spmd(
        _get_nc(), _in_maps(x), core_ids=list(range(N_CORES)), **spmd_kwargs
    )
    s = sum(float(np.sum(res.results[m]["partial"])) for m in range(N_CORES))
    loss = np.float32(-0.5 * s / B)
    return np.asarray(loss, dtype=np.float32), res


def kernel(student_output: np.ndarray) -> np.ndarray:
    x = np.ascontiguousarray(np.asarray(student_output, dtype=np.float32))
    loss, _ = run_kernel(x)
    return loss



# revision 21
# speedup vs baseline: 1.0113x; 1.0113x over previous
"""KoLeo loss kernel for Trainium2 (8 NeuronCores, data-parallel rows).

reference semantics:
    x = l2_normalize(student_output)            # [B, D]
    dots = x @ x.T ; dots[i, i] = -1
    I = argmax(dots, 1)
    loss = -mean(log(||x - x[I] + eps|| + eps))

Since rows are unit-norm, ||x_i - x_j|| = sqrt(2 - 2 * dot(x_i, x_j)), so
    loss = -0.5 * mean(ln(2 - 2 * max_j!=i dots[i, j]))
(the eps terms contribute ~1e-8 relative and are dropped).

Sharding: each core gets the full x^T (bf16, host-cast), column-rotated so its
own 1024 rows come first, computes its [1024, 8192] slice of the gram matrix
in fp8-e4m3 with DoubleRow matmuls (2 K-planes per instruction), and reduces
to a scalar partial sum of ln(2 - 2*maxdot).  Host sums the 8 partials.

Engine budget per core (all three within ~2us of each other, ~81us busy):
  PE : 256 DR gram matmuls + 32 presummed norm matmuls + diag-kill + warmup
  DVE: xsq + presum adds + bf16*bf16->fp8 normalize muls (fp8-out is ~2x
       slower than bf16-out; fp8 *reads* are also ~2x, so the input stays
       bf16) + 14 exact reduce_max drains
  ACT: 52 fused Exp smooth-max drains (accum_out row sums) + ln/exp norms

Hard-won scheduling rules baked in below:
  - GpSimd is NEVER used for tensor work: any big GP op degrades concurrent
    DVE throughput ~2.5x (measured 1136 -> 2830 ns per [128,1024] mul).
  - xsq pairs are presummed on DVE so the column-norm ones-matmul contracts
    2 planes instead of 4 (PE 17us -> 4us).
  - Direct (exact reduce_max, DVE) drains are tiles mi<2 of groups 3-7
    plus mi>=4 of the last group: spread evenly across group windows, early
    in each group so they never queue behind the next group's normalize
    muls in the DVE FIFO, and the closing drain wave lands on idle DVE.
  - 24+24 identity warmup matmuls bracket norm_stage(0): PE pstate drops to
    half clock after ~3us idle, and the first gram otherwise pays 2x.
  - group-0 input DMA is split into [32,1024] slices across all 16 queues
    (descriptor-rate-bound), the rest ride as 2048-wide 2KB-granule slices.

Other structure per 1024-wide column group:
  1. column norms via bf16 ones-matmul (PE broadcasts sums across
     partitions); inv = exp(-0.5*ln(norm2)) (ACT; the act-table universe is
     pinned to natural_log_exp_and_others => exactly one ACT_TABLE_LOAD)
  2. normalize: xq = x * inv, bf16*bf16 -> fp8 DoubleRow planes (DVE)
  3. gram: per row-tile one [128,1024] PSUM tile, 4 DoubleRow matmuls
     (K=256 each); diagonal killed by one extra bf16 I.T@(-64 shifted)
  4. drain: ACT smooth-max exp(beta*(dot-c)) with accum_out row sums
     (beta=256, c=0.22: error ~1e-4 vs exact; exp(256*(-64-c)) flushes the
     diagonal to zero for free), except the 12 direct-DVE tiles; epilogue
     folds smooth+direct maxes, ln(2-2*max) with a fused accum_out row-sum,
     then a partition-sum via f32 ones-matmul; the epilogue is split by
     row-tile halves so half A overlaps the last drain wave (host sums the
     [1,2] partials)
"""

import numpy as np
import ml_dtypes

import concourse.bacc as bacc
import concourse.hw_specs as hw_specs
import concourse.tile as tile
from concourse import mybir
from concourse.bass_utils import run_bass_kernel_spmd

B, D = 8192, 512
N_CORES = 8
ROWS = B // N_CORES          # 1024 rows per core
P = 128                      # SBUF partitions
KT = D // P                  # 4 contraction k-tiles
KB = KT // 2                 # 2 DoubleRow k-blocks (2 planes each)
M_TILES = ROWS // P          # 8 output row tiles
NT = 512                     # matmul moving free dim (psum bank)
GW = 1024                    # column-group width == gram PSUM tile (2 banks)
NG = B // GW                 # 8 column groups
DIAG_C = 64.0                # diagonal kill constant
N_WARM = 36                  # PE warm-up matmuls issued under the input DMA

BETA = 256.0                 # smooth-max sharpness
CSHIFT = 0.22                # smooth-max recentering constant
# Drain engine per column group: 3 of 8 groups go to exact DVE reduce_max,
# the rest to the ACT smooth-max.  Whole-group assignment keeps each engine
# FIFO free of cross-group head-of-line blocking (per-tile interleaving was
# measured slower: ready drains queue behind muls that wait on the next
# group's Exp).
DIRECT_SET = (2, 3, 4, 5, 6, 7)  # see _drain_direct
# Last group: drains alternate DVE/ACT per row-tile so the final drain wave
# finishes ~2x sooner (nothing queues behind them, so no FIFO hazard).
N_DIRECT = 6
N_EXP = NG


def _drain_direct(mi, ng):
    # 2 direct tiles per group, EARLY in the group (before the next group's
    # normalize muls enter the DVE queue), across groups 3-7: spreads the
    # DVE drain load evenly; the last group's final two tiles also go direct
    # so the closing drain wave lands on the by-then-idle DVE, not the
    # backlogged ACT
    return (ng >= 3 and mi < 2) or (ng == NG - 1 and mi >= 4)


def _drain_idx(mi, ng):
    """Index of this (mi, ng) tile within its row's direct or exp slots."""
    kind = _drain_direct(mi, ng)
    return sum(
        1 for n in range(ng) if _drain_direct(mi, n) == kind
    )

F32 = mybir.dt.float32
BF16 = mybir.dt.bfloat16
FP8 = mybir.dt.float8e4
AF = mybir.ActivationFunctionType
ALU = mybir.AluOpType
DR = mybir.MatmulPerfMode.DoubleRow

_CACHE: dict = {}


def _pin_act_tables():
    """Restrict the activation-table universe to natural_log_exp_and_others
    (contains ln, exp, copy, square, identity) so the table-load inserter
    emits exactly one ACT_TABLE_LOAD instead of thrashing ln<->exp sets.
    Set positions are preserved so the emitted act_func_set_id still indexes
    act_info.json correctly."""
    orig = hw_specs.get_activation_tables("gen3")
    pinned = {
        name: (fns if name == "natural_log_exp_and_others" else set())
        for name, fns in orig.items()
    }
    bacc.get_activation_tables = lambda arch: pinned


def _build():
    _pin_act_tables()
    nc = bacc.Bacc(
        "TRN2", target_bir_lowering=False, debug=False, num_devices=N_CORES
    )
    xt = nc.declare_dram_parameter("xt", [D, B], BF16, isOutput=False)
    ident = nc.declare_dram_parameter("ident", [P, P], BF16, isOutput=False)
    # ebig[p, 384 + p] = -DIAG_C, zero elsewhere; slicing [384-off : 896-off]
    # yields a [P, NT] tile with -DIAG_C at [p, off + p]
    ebig = nc.declare_dram_parameter("ebig", [P, NT + 3 * P], BF16, isOutput=False)
    partial = nc.declare_dram_parameter("partial", [1, 2], F32, isOutput=True)

    with tile.TileContext(nc) as tc:
        with (
            tc.tile_pool(name="big", bufs=1) as big,
            tc.tile_pool(name="work", bufs=2) as work,
            tc.tile_pool(name="sq", bufs=4) as sqp,
            tc.tile_pool(name="scr", bufs=5) as scrp,
            tc.tile_pool(name="small", bufs=2) as small,
        ):
            ident_sb = big.tile([P, P], BF16, name="ident_sb", tag="ident_sb")
            ebig_sb = big.tile([P, NT + 3 * P], BF16, name="ebig_sb", tag="ebig_sb")
            ones_sb = big.tile([P, P], BF16, name="ones_sb", tag="ones_sb")
            onesf_sb = big.tile([P, 1], F32, name="onesf_sb", tag="onesf_sb")
            nc.sync.dma_start(ident_sb[:], ident[:])
            nc.sync.dma_start(ebig_sb[:], ebig[:])
            nc.gpsimd.memset(ones_sb[:], 1.0)
            nc.gpsimd.memset(onesf_sb[:], 1.0)
            two_sb = small.tile([P, 1], F32, name="two_sb", tag="two_sb")
            nc.gpsimd.memset(two_sb[:], 2.0)
            # exp bias: exp(BETA*dot - BETA*CSHIFT)
            ebias_sb = small.tile([P, 1], F32, name="ebias_sb", tag="ebias_sb")
            nc.gpsimd.memset(ebias_sb[:], -BETA * CSHIFT)

            x8 = [
                big.tile([P, B], BF16, name=f"x8{k}", tag=f"x8{k}")
                for k in range(KT)
            ]
            xq = [
                big.tile([P, 2, B], FP8, name=f"xq{kb}", tag=f"xq{kb}")
                for kb in range(KB)
            ]
            invb = big.tile([P, B], BF16, name="invb", tag="invb")
            loglist = small.tile([P, M_TILES], F32, name="loglist", tag="loglist")
            # per (mi, direct slot) row-max candidates; 3D so the epilogue
            # can reduce all row-tiles in one instruction
            maxall = small.tile(
                [P, M_TILES, N_DIRECT], F32, name="maxall", tag="maxall"
            )
            # per (mi, exp slot) partial exp-sums
            expall = small.tile(
                [P, M_TILES, N_EXP], F32, name="expall", tag="expall"
            )
            # throwaway broadcast target for the fused drain's elementwise out
            dump = small.tile([P, 1], BF16, name="dump", tag="dump")
            # the last-group drain split leaves some slots unwritten
            nc.gpsimd.memset(maxall[:], -30000.0)
            nc.gpsimd.memset(expall[:], 0.0)

            with (
                tc.tile_pool(name="npsum", bufs=1, space="PSUM") as npsum,
                tc.tile_pool(name="gpsum", bufs=3, space="PSUM") as gpsum,
            ):
                # group 0 split across 16 queues (fp8 rows are 1KB granules,
                # descriptor-rate-bound; parallelism compensates), the rest as
                # 2KB-granule 2048-wide slices
                for k in range(KT):
                    for q in range(2):
                        r = slice(k * P + 64 * q, k * P + 64 * q + 64)
                        nc.sync.dma_start(x8[k][64 * q : 64 * q + 64, 0:GW], xt[r, 0:GW])
                for g0 in (1, 3, 5):
                    for k in range(KT):
                        nc.sync.dma_start(
                            x8[k][:, g0 * GW : (g0 + 2) * GW],
                            xt[k * P : (k + 1) * P, g0 * GW : (g0 + 2) * GW],
                        )
                for k in range(KT):
                    nc.sync.dma_start(
                        x8[k][:, 7 * GW : 8 * GW], xt[k * P : (k + 1) * P, 7 * GW : 8 * GW]
                    )

                # PE warm-up: keep the HAM activity window busy during the
                # initial DMA so gram matmuls run at 2.4 GHz from the start.
                warm = gpsum.tile([P, GW], F32, name="warm", tag="g")
                for _ in range(N_WARM):
                    nc.tensor.matmul(
                        warm[:, 0:P], ident_sb[:], ident_sb[:], start=True, stop=True
                    )

                def norm_stage(ng, startup=False):
                    """xsq -> ones-matmul -> ln/exp -> normalize for group ng.
                    Emitted one group ahead, mid-way through the previous
                    group's gram loop, so the PE FIFO interleaves the norm
                    matmuls with gram matmuls instead of serializing at the
                    group boundary."""
                    ns = slice(ng * GW, (ng + 1) * GW)
                    # GpSimd is kept OFF this path entirely: its slow
                    # [128,1024] ops degrade concurrent DVE throughput ~2.5x
                    xsq = [
                        sqp.tile([P, GW], BF16, name=f"xsq_{ng}_{k}", tag=f"xsq{k}")
                        for k in range(KT)
                    ]
                    for k in range(KT):
                        nc.vector.tensor_mul(xsq[k][:], x8[k][:, ns], x8[k][:, ns])
                    # pre-sum pairs so the ones-matmul contracts 2 planes not 4
                    if startup:
                        # group 0 is latency-critical: feed the ones-matmul
                        # straight from the 4 xsq planes (no presum hop) and
                        # emit the normalize muls in 512-halves so the first
                        # gram chunk starts after half the mul work
                        nps = npsum.tile([P, GW], F32, name="nps", tag="nps")
                        for c in range(GW // NT):
                            cs = slice(c * NT, (c + 1) * NT)
                            for k in range(KT):
                                nc.tensor.matmul(
                                    nps[:, cs], ones_sb[:], xsq[k][:, cs],
                                    start=(k == 0), stop=(k == KT - 1),
                                )
                        lntmp = work.tile([P, GW], F32, name="lntmp", tag="lntmp")
                        nc.scalar.activation(lntmp[:], nps[:], AF.Ln)
                        nc.scalar.activation(invb[:, ns], lntmp[:], AF.Exp, scale=-0.5)
                        for h in range(2):
                            hs = slice(1024 * ng + NT * h, 1024 * ng + NT * h + NT)
                            for k in range(KT):
                                nc.vector.tensor_mul(
                                    xq[k // 2][:, k % 2, hs], x8[k][:, hs], invb[:, hs]
                                )
                        return
                    x01 = sqp.tile([P, GW], BF16, name=f"x01_{ng}", tag="x01")
                    x23 = sqp.tile([P, GW], BF16, name=f"x23_{ng}", tag="x23")
                    nc.vector.tensor_add(x01[:], xsq[0][:], xsq[1][:])
                    nc.vector.tensor_add(x23[:], xsq[2][:], xsq[3][:])
                    nps = npsum.tile([P, GW], F32, name="nps", tag="nps")
                    for c in range(GW // NT):
                        cs = slice(c * NT, (c + 1) * NT)
                        nc.tensor.matmul(
                            nps[:, cs], ones_sb[:], x01[:, cs], start=True, stop=False
                        )
                        nc.tensor.matmul(
                            nps[:, cs], ones_sb[:], x23[:, cs], start=False, stop=True
                        )
                    # inv = exp(-0.5*ln(norm2)); one pinned table set
                    lntmp = work.tile([P, GW], F32, name="lntmp", tag="lntmp")
                    nc.scalar.activation(lntmp[:], nps[:], AF.Ln)
                    nc.scalar.activation(
                        invb[:, ns], lntmp[:], AF.Exp, scale=-0.5
                    )
                    # normalize into fp8 DoubleRow planes: xq = x * inv
                    # (single op; a bf16-mul + fp8-copy split was measured
                    # slower despite the fp8-out 1x rate)
                    for k in range(KT):
                        nc.vector.tensor_mul(
                            xq[k // 2][:, k % 2, ns], x8[k][:, ns], invb[:, ns]
                        )

                norm_stage(0, startup=True)
                for _ in range(24):
                    nc.tensor.matmul(
                        warm[:, 0:P], ident_sb[:], ident_sb[:], start=True, stop=True
                    )
                for ng in range(NG):
                    # gram slice rows x this column group, then row-max drain
                    for mi in range(M_TILES):
                        if mi == 3 and ng + 1 < NG:
                            norm_stage(ng + 1)
                        g = gpsum.tile([P, GW], F32, name="g", tag="g")
                        # diag block for row-tile mi sits at columns
                        # [mi*128, mi*128+128) -- always group 0
                        diag_here = ng == 0
                        diag_c = (mi * P) // NT
                        for kb in range(KB):
                            for c in range(GW // NT):
                                c0 = ng * GW + c * NT
                                nc.tensor.matmul(
                                    g[:, c * NT : (c + 1) * NT],
                                    xq[kb][:, :, mi * P : (mi + 1) * P],
                                    xq[kb][:, :, c0 : c0 + NT],
                                    start=(kb == 0),
                                    stop=(
                                        kb == KB - 1
                                        and not (diag_here and c == diag_c)
                                    ),
                                    perf_mode=DR,
                                )
                        if diag_here:
                            off = (mi * P) % NT
                            # adds -DIAG_C at diag position [p, off+p]
                            nc.tensor.matmul(
                                g[:, diag_c * NT : (diag_c + 1) * NT],
                                ident_sb[:],
                                ebig_sb[:, 3 * P - off : 3 * P - off + NT],
                                start=False,
                                stop=True,
                            )
                        si = _drain_idx(mi, ng)
                        if _drain_direct(mi, ng):
                            nc.vector.reduce_max(
                                maxall[:, mi, si : si + 1],
                                g[:],
                                axis=mybir.AxisListType.X,
                            )
                        else:
                            # fused smooth-max drain on ACT: accum_out sums
                            # exp(BETA*(dot - CSHIFT)) along the row
                            scr = scrp.tile([P, GW], BF16, name="scr", tag="scr")
                            nc.scalar.activation(
                                scr[:],
                                g[:],
                                AF.Exp,
                                bias=ebias_sb[:],
                                scale=BETA,
                                accum_out=expall[:, mi, si : si + 1],
                            )

# --- epilogue, split by row-tile halves: mi 0-3 finish their last
                # drain ~3us before mi 4-7, so half A's serial chain overlaps
                # the closing drain wave; host sums the two partials.
                # S >= exp(beta*(min-max-dot-c)) ~ 1e-10 here, no clamp needed
                tot = npsum.tile([P, GW], F32, name="tot", tag="nps")
                for hh in range(2):
                    sl = slice(4 * hh, 4 * hh + 4)
                    acc = small.tile([P, 4], F32, name=f"acc{hh}", tag=f"acc{hh}")
                    nc.vector.reduce_sum(
                        acc[:], expall[:, sl, :], axis=mybir.AxisListType.X
                    )
                    lnacc = small.tile([P, 4], F32, name=f"lnacc{hh}", tag=f"lnacc{hh}")
                    nc.scalar.activation(lnacc[:], acc[:], AF.Ln)
                    smooth = small.tile([P, 4], F32, name=f"smooth{hh}", tag=f"smooth{hh}")
                    nc.vector.tensor_scalar(
                        smooth[:],
                        lnacc[:],
                        1.0 / BETA,
                        CSHIFT,
                        op0=ALU.mult,
                        op1=ALU.add,
                    )
                    dmax = small.tile([P, 4], F32, name=f"dmax{hh}", tag=f"dmax{hh}")
                    nc.vector.reduce_max(
                        dmax[:], maxall[:, sl, :], axis=mybir.AxisListType.X
                    )
                    rowmax = small.tile([P, 4], F32, name=f"rowmax{hh}", tag=f"rowmax{hh}")
                    nc.vector.tensor_max(rowmax[:], dmax[:], smooth[:])
                    sumlog = small.tile([P, 1], F32, name=f"sumlog{hh}", tag=f"sumlog{hh}")
                    nc.scalar.activation(
                        loglist[:, sl], rowmax[:], AF.Ln, bias=two_sb[:],
                        scale=-2.0, accum_out=sumlog[:],
                    )
                    nc.tensor.matmul(
                        tot[0:1, hh : hh + 1], sumlog[:], onesf_sb[:],
                        start=True, stop=True,
                    )
                part_sb = small.tile([1, 2], F32, name="part_sb", tag="part_sb")
                nc.vector.tensor_copy(part_sb[:], tot[0:1, 0:2])
                nc.sync.dma_start(partial[:], part_sb[:])

    nc.finalize()
    return nc


def _get_nc():
    if "nc" not in _CACHE:
        _CACHE["nc"] = _build()
    return _CACHE["nc"]


def _in_maps(x: np.ndarray) -> list[dict]:
    ident = np.eye(P, dtype=np.float32).astype(ml_dtypes.bfloat16)
    ebig = np.zeros((P, NT + 3 * P), dtype=np.float32)
    ebig[np.arange(P), 3 * P + np.arange(P)] = -DIAG_C
    ebig = ebig.astype(ml_dtypes.bfloat16)
    x8 = x.astype(ml_dtypes.bfloat16)
    maps = []
    for m in range(N_CORES):
        xrot = np.concatenate([x8[m * ROWS :], x8[: m * ROWS]], axis=0)
        maps.append(
            {
                "xt": np.ascontiguousarray(xrot.T),
                "ident": ident,
                "ebig": ebig,
            }
        )
    return maps


def run_kernel(x: np.ndarray, **spmd_kwargs):
    """Returns (loss_scalar_f32, BassKernelResults)."""
    res = run_bass_kernel_spmd(
        _get_nc(), _in_maps(x), core_ids=list(range(N_CORES)), **spmd_kwargs
    )
    s = sum(float(np.sum(res.results[m]["partial"])) for m in range(N_CORES))
    loss = np.float32(-0.5 * s / B)
    return np.asarray(loss, dtype=np.float32), res


def kernel(student_output: np.ndarray) -> np.ndarray:
    x = np.ascontiguousarray(np.asarray(student_output, dtype=np.float32))
    loss, _ = run_kernel(x)
    return loss

